# revision 15
# baseline (speedup 1.0000x reference)
"""Trainium2 Bass kernel for nn_DeformConv2d_72765335929324.

The module is a dense 3x3 conv (stride 1, pad 1) [B,64,256,256] -> [B,18,256,256]
plus a per-pixel additive `offset` term and a channel bias.

Strategy (per core; batch is sharded 2 images/core across 8 cores):
- conv = 9 taps, each a [cin=64 -> cout=18] matmul over shifted input views.
- PE array packing via tile_position: 2 images on row-groups {0,64} x 4
  output row-pair chunks on col-groups {0,32,64,96} -> 8 concurrent matmul
  streams, 9 accumulating taps each, N=512 (2 output rows) per stream.
- offset+bias are pre-added AND pre-permuted to the SBUF tile layout on the
  host, so the device does one flat [128, 2048] DMA per (image, 32-row slab)
  instead of many small strided ones (small DMAs serialize at ~0.75us each
  on this stack). The output leaves in the same packed layout (incl. the 14
  pad channels per col group) and is un-permuted on the host.
- W-edge zero-padding is realized by shrinking the matmul N-span per kw tap;
  H-edge padding by shrinking the row span of the first/last chunk taps.
"""

import contextlib
import os
import numpy as np

import concourse.bass as bass
import concourse.tile as tile
import concourse.mybir as mybir
from concourse.vector_clock import ScopedClock
from concourse.bass_utils import run_bass_kernel_spmd

B, CIN, H, W = 16, 64, 256, 256
COUT = 18
COUTP = 32  # cout padded to a 32-wide PE column group
NCORES = 8
BPC = B // NCORES  # images per core
R = 64  # output rows per input slab
NSLAB = H // R
QPS = R // 8  # quad chunks per slab (each quad = 8 rows)
FQ = 2 * W  # free size of one quad chunk (2 rows x 256 cols)
FS = QPS * FQ  # free size of one slab tile

# tap order: full-coverage center tap first so start=True initializes the
# whole PSUM bank region before partial-coverage taps accumulate.
TAPS = [(1, 1), (0, 0), (0, 1), (0, 2), (1, 0), (1, 2), (2, 0), (2, 1), (2, 2)]

# kw -> (src col offset, dst col offset, ncols): zero-pad at W edges is
# realized by shrinking the span instead of padding SBUF.
KW_SPAN = {0: (0, 1, W - 1), 1: (0, 0, W), 2: (1, 0, W - 1)}

DT_NAME = os.environ.get("CONV_DT", "bfloat16")
# timing experiments only: restrict the tap count (wrong results!)
N_TAPS = int(os.environ.get("CONV_TAPS", "9"))


class _TileContext(tile.TileContext):
    """TileContext whose tail drain spreads its semaphore waits over NOPs.

    The stock _drain_and_barrier puts one wait per logical proc on a single
    Drain instruction; the walrus build here rejects instructions carrying
    more than 1-2 sync waits.
    """

    def _drain_and_barrier(self, tick_clock, wait_clock):
        nc = self.nc
        carriers = [nc.sync.nop(nofuse=True) for _ in range(64)]
        drain_inst = nc.sync.drain()
        wait_clock.add_sem_waits(
            drain_inst.ins, ScopedClock({None: tick_clock.global_clock})
        )
        si = drain_inst.ins.sync_info
        waits = list(si.on_wait or []) if si is not None else []
        if len(waits) > 1:
            si.on_wait = waits[:1]
            extra = waits[1:]
            assert len(extra) <= len(carriers)
            for wt, nop in zip(extra, carriers):
                nsi = nop.ins.sync_info
                if nsi is None:
                    nop.ins.sync_info = mybir.SyncInfo(on_wait=[wt], on_update=[])
                else:
                    nsi.on_wait = [wt]
        nc.all_engine_barrier()
        assert self.sems is not None
        popped = nc._tile_sem_poison_stack.pop()
        assert popped is self._sem_poison
        nc.clear_and_free_semaphores(list(self.sems.allocated().values()))
        nc.all_engine_barrier()


def _split_excess_waits(nc):
    """Spill per-instruction semaphore waits onto same-engine NOP carriers.

    Tile's wait assigner attaches up to ~6 waits to one instruction; the
    walrus build here rejects >1 sync wait on engine instructions (>2 on
    EventSemaphore). A NOP that runs just before the instruction on the same
    engine is semantically equivalent (program order on one engine is
    serial). For DMAs, the wait kept in-descriptor is evaluated by the DGE
    without stalling the issuing engine, so keep the freshest (engine-sem)
    wait there and spill the long-satisfied WAR waits on old DMA completions.
    """
    for bb in nc.m.functions[0].blocks:
        new = []
        for inst in bb.instructions:
            si = inst.sync_info
            waits = list(si.on_wait) if si and si.on_wait else []
            cap = 2 if isinstance(inst, mybir.InstEventSemaphore) else 1
            if len(waits) > cap:
                if isinstance(inst, mybir.InstDMACopy):
                    waits.sort(key=lambda w: ((w.ant_name or "").startswith("DMA"),))
                si.on_wait = waits[:cap]
                for w in waits[cap:]:
                    n = mybir.InstNoOp(
                        name=nc.get_next_instruction_name(), ins=[], outs=[]
                    )
                    n.engine = inst.engine
                    n.sync_info = mybir.SyncInfo(on_wait=[w], on_update=[])
                    new.append(n)
            new.append(inst)
        bb.instructions = new


def build_nc(dt_name=DT_NAME, h=H, reps=1):
    dt_in = getattr(mybir.dt, dt_name)
    f32 = mybir.dt.float32
    nslab = h // R
    nc = bass.Bass()
    x = nc.dram_tensor("x", [BPC, CIN, h, W], dt_in, kind="ExternalInput")
    off = nc.dram_tensor("off", [BPC, nslab, 128, FS], f32, kind="ExternalInput")
    wt = nc.dram_tensor("w", [128, len(TAPS) * COUTP], dt_in, kind="ExternalInput")
    y = nc.dram_tensor("y", [BPC, nslab, 128, FS], f32, kind="ExternalOutput")

    nb = 3 if mybir.dt.size(dt_in) <= 2 else 2  # SBUF budget: fp32 input needs
    with _TileContext(nc) as tc:                 # shallower pools
        with (
            tc.tile_pool(name="wpool", bufs=1) as wpool,
            tc.tile_pool(name="slabp", bufs=nb) as slabp,
            tc.tile_pool(name="offp", bufs=nb) as offp,
            tc.tile_pool(name="outp", bufs=nb) as outp,
            tc.tile_pool(name="psump", bufs=6, space="PSUM") as psump,
        ):
            w_t = wpool.tile([128, len(TAPS) * COUTP], dt_in, name="w_t")
            nc.sync.dma_start(w_t[:, :], wt[:, :])

            loop_ctx = tc.For_i(0, reps) if reps > 1 else contextlib.nullcontext()
            with loop_ctx:
                for s in range(nslab):
                    # slab slot j <-> input row s*R - 1 + j (34 slots w/ halo)
                    slab = slabp.tile([128, (R + 2) * W], dt_in, name="slab")
                    r_lo = max(0, s * R - 1)
                    r_hi = min(h, s * R + R + 1)
                    slot0 = r_lo - (s * R - 1)
                    for img in range(BPC):
                        nc.sync.dma_start(
                            slab[
                                img * 64 : (img + 1) * 64,
                                slot0 * W : (slot0 + (r_hi - r_lo)) * W,
                            ],
                            x[img, :, r_lo:r_hi, :],
                        )
                    slab3 = [
                        slab[img * 64 : (img + 1) * 64, :].rearrange(
                            "p (r w) -> p r w", w=W
                        )
                        for img in range(BPC)
                    ]

                    offts, outts = [], []
                    for img in range(BPC):
                        off_t = offp.tile([128, FS], f32, name="off_t")
                        nc.sync.dma_start(off_t[:, :], off[img, s, :, :])
                        out_t = outp.tile([128, FS], f32, name="out_t")
                        offts.append(off_t)
                        outts.append(out_t)

                    for q in range(QPS):
                        psums = []
                        for img in range(BPC):
                            psum_t = psump.tile([128, FQ], f32, name="psum_t")
                            psums.append(psum_t)

                        # t-major emission: 8 streams (4 col-groups x 2
                        # images) advance through the taps in lockstep.
                        for ti, (kh, kw) in enumerate(TAPS[:N_TAPS] if N_TAPS else []):
                            for c in range(4):
                                for img in range(BPC):
                                    r0 = q * 8 + 2 * c
                                    gr0 = s * R + r0
                                    row_lo, nrows = 0, 2
                                    if gr0 == 0 and kh == 0:
                                        row_lo, nrows = 1, 1
                                    if gr0 == h - 2 and kh == 2:
                                        nrows = 1
                                    src_off, dst_off, ncol = KW_SPAN[kw]
                                    slot = r0 + row_lo + kh
                                    rhs = slab3[img][
                                        :, slot : slot + nrows, src_off : src_off + ncol
                                    ]
                                    out_ap = psums[img][
                                        32 * c : 32 * c + COUTP, :
                                    ].rearrange("p (r w) -> p r w", w=W)[
                                        :,
                                        row_lo : row_lo + nrows,
                                        dst_off : dst_off + ncol,
                                    ]
                                    lhsT = w_t[
                                        img * 64 : (img + 1) * 64,
                                        ti * COUTP : (ti + 1) * COUTP,
                                    ]
                                    nc.tensor.matmul(
                                        out_ap,
                                        lhsT,
                                        rhs,
                                        start=(ti == 0),
                                        stop=(ti == N_TAPS - 1),
                                        tile_position=(img * 64, 32 * c),
                                        # the sim's accumulation-group sanity
                                        # check mis-addresses partition-sliced
                                        # PSUM groups; its per-element
                                        # pending-zero modeling stays active.
                                        skip_group_check=True,
                                    )

                        for img in range(BPC):
                            dst = outts[img][:, q * FQ : (q + 1) * FQ]
                            if N_TAPS:
                                nc.vector.tensor_add(
                                    dst,
                                    psums[img][:, :],
                                    offts[img][:, q * FQ : (q + 1) * FQ],
                                )
                            else:
                                nc.vector.tensor_copy(
                                    dst, offts[img][:, q * FQ : (q + 1) * FQ]
                                )

                    for img in range(BPC):
                        nc.sync.dma_start(y[img, s, :, :], outts[img][:, :])
    _split_excess_waits(nc)
    return nc


def _pack_off(offb, h):
    """[n, 32, h, W] -> [n, nslab, 128, FS] in the SBUF tile layout.

    row r = s*32 + q*8 + c*2 + rw maps to partition c*32+ch, free
    q*512 + rw*256 + w.
    """
    nslab = h // R
    v = offb.reshape(offb.shape[0], COUTP, nslab, QPS, 4, 2, W)
    v = v.transpose(0, 2, 4, 1, 3, 5, 6)  # n, s, c, ch, q, rw, w
    return np.ascontiguousarray(v.reshape(offb.shape[0], nslab, 128, FS))


def _unpack_y(y_dev, h):
    """[n, nslab, 128, FS] packed -> [n, COUT, h, W]."""
    n = y_dev.shape[0]
    nslab = h // R
    v = y_dev.reshape(n, nslab, 4, COUTP, QPS, 2, W)
    v = v.transpose(0, 3, 1, 4, 2, 5, 6)  # n, ch, s, q, c, rw, w
    return v.reshape(n, COUTP, h, W)[:, :COUT]


def pack_inputs(input, offset, weight, bias, dt_name=DT_NAME, h=H):
    np_in = mybir.dt.np(getattr(mybir.dt, dt_name))
    input = np.asarray(input, dtype=np.float32)
    offset = np.asarray(offset, dtype=np.float32)
    weight = np.asarray(weight, dtype=np.float32)
    bias = np.asarray(bias, dtype=np.float32)

    nimg = input.shape[0]
    offb = np.zeros((nimg, COUTP, h, W), dtype=np.float32)
    offb[:, :COUT] = offset[:, :COUT, :h] + bias[None, :, None, None]
    off_packed = _pack_off(offb, h)
    w_packed = np.zeros((128, len(TAPS) * COUTP), dtype=np_in)
    for t, (kh, kw) in enumerate(TAPS):
        w_packed[0:64, t * COUTP : t * COUTP + COUT] = weight[:, :, kh, kw].T.astype(
            np_in
        )
    w_packed[64:128] = w_packed[0:64]
    xc = input.astype(np_in)
    in_maps = [
        {
            "x": np.ascontiguousarray(xc[BPC * k : BPC * (k + 1), :, :h]),
            "off": off_packed[BPC * k : BPC * (k + 1)],
            "w": w_packed,
        }
        for k in range(NCORES)
    ]
    return in_maps


_NC_CACHE = {}


def run_on_hw(input, offset, weight, bias, dt_name=DT_NAME, trace=False):
    key = dt_name
    if key not in _NC_CACHE:
        _NC_CACHE[key] = build_nc(dt_name)
    nc = _NC_CACHE[key]
    in_maps = pack_inputs(input, offset, weight, bias, dt_name)
    res = run_bass_kernel_spmd(nc, in_maps, list(range(NCORES)), trace=trace)
    y_dev = np.concatenate([res.results[k]["y"] for k in range(NCORES)], axis=0)
    out = _unpack_y(y_dev, H)
    return np.ascontiguousarray(out.astype(np.float32, copy=False)), res


def kernel(input, offset, weight, bias):
    out, _ = run_on_hw(input, offset, weight, bias)
    return out


# revision 19
# speedup vs baseline: 37961.4214x; 37961.4214x over previous
"""Trainium2 Bass kernel for nn_DeformConv2d_72765335929324.

The module is a dense 3x3 conv (stride 1, pad 1) [B,64,256,256] -> [B,18,256,256]
plus a per-pixel additive `offset` term and a channel bias.

Strategy (per core; batch is sharded 2 images/core across 8 cores):
- conv = 9 taps, each a [cin=64 -> cout=18] matmul over shifted input views.
- PE array packing via tile_position: 2 images on row-groups {0,64} x 4
  output row-pair chunks on col-groups {0,32,64,96} -> 8 concurrent matmul
  streams, 9 accumulating taps each, N=512 (2 output rows) per stream.
- offset+bias are pre-added AND pre-permuted to the SBUF tile layout on the
  host, so the device does one flat [128, 2048] DMA per (image, 32-row slab)
  instead of many small strided ones (small DMAs serialize at ~0.75us each
  on this stack). The output leaves in the same packed layout (incl. the 14
  pad channels per col group) and is un-permuted on the host.
- W-edge zero-padding is realized by shrinking the matmul N-span per kw tap;
  H-edge padding by shrinking the row span of the first/last chunk taps.
"""

import contextlib
import os
import numpy as np

import concourse.bass as bass
import concourse.tile as tile
import concourse.mybir as mybir
from concourse.vector_clock import ScopedClock
from concourse.bass_utils import run_bass_kernel_spmd

B, CIN, H, W = 16, 64, 256, 256
COUT = 18
COUTP = 32  # cout padded to a 32-wide PE column group
NCORES = 8
BPC = B // NCORES  # images per core
R = 64  # output rows per input slab
NSLAB = H // R
QPS = R // 8  # quad chunks per slab (each quad = 8 rows)
FQ = 2 * W  # free size of one quad chunk (2 rows x 256 cols)
FS = QPS * FQ  # free size of one slab tile

# tap order: full-coverage center tap first so start=True initializes the
# whole PSUM bank region before partial-coverage taps accumulate.
TAPS = [(1, 1), (0, 0), (0, 1), (0, 2), (1, 0), (1, 2), (2, 0), (2, 1), (2, 2)]

# kw -> (src col offset, dst col offset, ncols): zero-pad at W edges is
# realized by shrinking the span instead of padding SBUF.
KW_SPAN = {0: (0, 1, W - 1), 1: (0, 0, W), 2: (1, 0, W - 1)}

DT_NAME = os.environ.get("CONV_DT", "bfloat16")
# timing experiments only: restrict the tap count (wrong results!)
N_TAPS = int(os.environ.get("CONV_TAPS", "9"))


class _TileContext(tile.TileContext):
    """TileContext whose tail drain spreads its semaphore waits over NOPs.

    The stock _drain_and_barrier puts one wait per logical proc on a single
    Drain instruction; the walrus build here rejects instructions carrying
    more than 1-2 sync waits.
    """

    def _drain_and_barrier(self, tick_clock, wait_clock):
        nc = self.nc
        carriers = [nc.sync.nop(nofuse=True) for _ in range(64)]
        drain_inst = nc.sync.drain()
        wait_clock.add_sem_waits(
            drain_inst.ins, ScopedClock({None: tick_clock.global_clock})
        )
        si = drain_inst.ins.sync_info
        waits = list(si.on_wait or []) if si is not None else []
        if len(waits) > 1:
            si.on_wait = waits[:1]
            extra = waits[1:]
            assert len(extra) <= len(carriers)
            for wt, nop in zip(extra, carriers):
                nsi = nop.ins.sync_info
                if nsi is None:
                    nop.ins.sync_info = mybir.SyncInfo(on_wait=[wt], on_update=[])
                else:
                    nsi.on_wait = [wt]
        nc.all_engine_barrier()
        assert self.sems is not None
        popped = nc._tile_sem_poison_stack.pop()
        assert popped is self._sem_poison
        nc.clear_and_free_semaphores(list(self.sems.allocated().values()))
        nc.all_engine_barrier()


def _split_excess_waits(nc):
    """Spill per-instruction semaphore waits onto same-engine NOP carriers.

    Tile's wait assigner attaches up to ~6 waits to one instruction; the
    walrus build here rejects >1 sync wait on engine instructions (>2 on
    EventSemaphore). A NOP that runs just before the instruction on the same
    engine is semantically equivalent (program order on one engine is
    serial). For DMAs, the wait kept in-descriptor is evaluated by the DGE
    without stalling the issuing engine, so keep the freshest (engine-sem)
    wait there and spill the long-satisfied WAR waits on old DMA completions.
    """
    for bb in nc.m.functions[0].blocks:
        new = []
        for inst in bb.instructions:
            si = inst.sync_info
            waits = list(si.on_wait) if si and si.on_wait else []
            cap = 2 if isinstance(inst, mybir.InstEventSemaphore) else 1
            if len(waits) > cap:
                if isinstance(inst, mybir.InstDMACopy):
                    waits.sort(key=lambda w: ((w.ant_name or "").startswith("DMA"),))
                si.on_wait = waits[:cap]
                for w in waits[cap:]:
                    n = mybir.InstNoOp(
                        name=nc.get_next_instruction_name(), ins=[], outs=[]
                    )
                    n.engine = inst.engine
                    n.sync_info = mybir.SyncInfo(on_wait=[w], on_update=[])
                    new.append(n)
            new.append(inst)
        bb.instructions = new


def build_nc(dt_name=DT_NAME, h=H, reps=1):
    dt_in = getattr(mybir.dt, dt_name)
    f32 = mybir.dt.float32
    nslab = h // R
    nc = bass.Bass()
    x = nc.dram_tensor("x", [BPC, CIN, h, W], dt_in, kind="ExternalInput")
    off = nc.dram_tensor("off", [BPC, nslab, 128, FS], f32, kind="ExternalInput")
    wt = nc.dram_tensor("w", [128, len(TAPS) * COUTP], dt_in, kind="ExternalInput")
    y = nc.dram_tensor("y", [BPC, nslab, 128, FS], f32, kind="ExternalOutput")

    nb = 4 if mybir.dt.size(dt_in) <= 2 else 2  # SBUF budget: fp32 input needs
    with _TileContext(nc) as tc:                 # shallower pools
        with (
            tc.tile_pool(name="wpool", bufs=1) as wpool,
            tc.tile_pool(name="slabp", bufs=2) as slabp,
            tc.tile_pool(name="offp", bufs=nb) as offp,
            tc.tile_pool(name="outp", bufs=nb) as outp,
            tc.tile_pool(name="psump", bufs=8, space="PSUM") as psump,
        ):
            w_t = wpool.tile([128, len(TAPS) * COUTP], dt_in, name="w_t")
            nc.sync.dma_start(w_t[:, :], wt[:, :])

            loop_ctx = tc.For_i(0, reps) if reps > 1 else contextlib.nullcontext()
            with loop_ctx:
                for s in range(nslab):
                    # slab slot j <-> input row s*R - 1 + j (34 slots w/ halo)
                    slab = slabp.tile([128, (R + 2) * W], dt_in, name="slab")
                    r_lo = max(0, s * R - 1)
                    r_hi = min(h, s * R + R + 1)
                    slot0 = r_lo - (s * R - 1)
                    nc.sync.dma_start(
                        slab[:, slot0 * W : (slot0 + (r_hi - r_lo)) * W],
                        x[:, :, r_lo:r_hi, :],
                    )
                    slab3 = [
                        slab[img * 64 : (img + 1) * 64, :].rearrange(
                            "p (r w) -> p r w", w=W
                        )
                        for img in range(BPC)
                    ]

                    offts, outts = [], []
                    for img in range(BPC):
                        off_t = offp.tile([128, FS], f32, name="off_t")
                        nc.sync.dma_start(off_t[:, :], off[img, s, :, :])
                        out_t = outp.tile([128, FS], f32, name="out_t")
                        offts.append(off_t)
                        outts.append(out_t)

                    for q in range(QPS):
                        psums = []
                        for img in range(BPC):
                            psum_t = psump.tile([128, FQ], f32, name="psum_t")
                            psums.append(psum_t)

                        # t-major emission: 8 streams (4 col-groups x 2
                        # images) advance through the taps in lockstep.
                        for ti, (kh, kw) in enumerate(TAPS[:N_TAPS] if N_TAPS else []):
                            for c in range(4):
                                for img in range(BPC):
                                    r0 = q * 8 + 2 * c
                                    gr0 = s * R + r0
                                    row_lo, nrows = 0, 2
                                    if gr0 == 0 and kh == 0:
                                        row_lo, nrows = 1, 1
                                    if gr0 == h - 2 and kh == 2:
                                        nrows = 1
                                    src_off, dst_off, ncol = KW_SPAN[kw]
                                    slot = r0 + row_lo + kh
                                    rhs = slab3[img][
                                        :, slot : slot + nrows, src_off : src_off + ncol
                                    ]
                                    out_ap = psums[img][
                                        32 * c : 32 * c + COUTP, :
                                    ].rearrange("p (r w) -> p r w", w=W)[
                                        :,
                                        row_lo : row_lo + nrows,
                                        dst_off : dst_off + ncol,
                                    ]
                                    lhsT = w_t[
                                        img * 64 : (img + 1) * 64,
                                        ti * COUTP : (ti + 1) * COUTP,
                                    ]
                                    nc.tensor.matmul(
                                        out_ap,
                                        lhsT,
                                        rhs,
                                        start=(ti == 0),
                                        stop=(ti == N_TAPS - 1),
                                        tile_position=(img * 64, 32 * c),
                                        # the sim's accumulation-group sanity
                                        # check mis-addresses partition-sliced
                                        # PSUM groups; its per-element
                                        # pending-zero modeling stays active.
                                        skip_group_check=True,
                                    )

                        for img in range(BPC):
                            dst = outts[img][:, q * FQ : (q + 1) * FQ]
                            if N_TAPS:
                                nc.vector.tensor_add(
                                    dst,
                                    psums[img][:, :],
                                    offts[img][:, q * FQ : (q + 1) * FQ],
                                )
                            else:
                                nc.vector.tensor_copy(
                                    dst, offts[img][:, q * FQ : (q + 1) * FQ]
                                )

                    for img in range(BPC):
                        nc.sync.dma_start(y[img, s, :, :], outts[img][:, :])
    _split_excess_waits(nc)
    return nc


def _pack_off(offb, h):
    """[n, 32, h, W] -> [n, nslab, 128, FS] in the SBUF tile layout.

    row r = s*R + q*8 + c*2 + rw maps to partition c*32+ch, free
    q*512 + rw*256 + w.
    """
    nslab = h // R
    v = offb.reshape(offb.shape[0], COUTP, nslab, QPS, 4, 2, W)
    v = v.transpose(0, 2, 4, 1, 3, 5, 6)  # n, s, c, ch, q, rw, w
    return np.ascontiguousarray(v.reshape(offb.shape[0], nslab, 128, FS))


def _unpack_y(y_dev, h):
    """[n, nslab, 128, FS] packed -> [n, COUT, h, W]."""
    n = y_dev.shape[0]
    nslab = h // R
    v = y_dev.reshape(n, nslab, 4, COUTP, QPS, 2, W)
    v = v.transpose(0, 3, 1, 4, 2, 5, 6)  # n, ch, s, q, c, rw, w
    return v.reshape(n, COUTP, h, W)[:, :COUT]


def pack_inputs(input, offset, weight, bias, dt_name=DT_NAME, h=H):
    np_in = mybir.dt.np(getattr(mybir.dt, dt_name))
    input = np.asarray(input, dtype=np.float32)
    offset = np.asarray(offset, dtype=np.float32)
    weight = np.asarray(weight, dtype=np.float32)
    bias = np.asarray(bias, dtype=np.float32)

    nimg = input.shape[0]
    offb = np.zeros((nimg, COUTP, h, W), dtype=np.float32)
    offb[:, :COUT] = offset[:, :COUT, :h] + bias[None, :, None, None]
    off_packed = _pack_off(offb, h)
    w_packed = np.zeros((128, len(TAPS) * COUTP), dtype=np_in)
    for t, (kh, kw) in enumerate(TAPS):
        w_packed[0:64, t * COUTP : t * COUTP + COUT] = weight[:, :, kh, kw].T.astype(
            np_in
        )
    w_packed[64:128] = w_packed[0:64]
    xc = input.astype(np_in)
    in_maps = [
        {
            "x": np.ascontiguousarray(xc[BPC * k : BPC * (k + 1), :, :h]),
            "off": off_packed[BPC * k : BPC * (k + 1)],
            "w": w_packed,
        }
        for k in range(NCORES)
    ]
    return in_maps


_NC_CACHE = {}


def run_on_hw(input, offset, weight, bias, dt_name=DT_NAME, trace=False):
    key = dt_name
    if key not in _NC_CACHE:
        _NC_CACHE[key] = build_nc(dt_name)
    nc = _NC_CACHE[key]
    in_maps = pack_inputs(input, offset, weight, bias, dt_name)
    res = run_bass_kernel_spmd(nc, in_maps, list(range(NCORES)), trace=trace)
    y_dev = np.concatenate([res.results[k]["y"] for k in range(NCORES)], axis=0)
    out = _unpack_y(y_dev, H)
    return np.ascontiguousarray(out.astype(np.float32, copy=False)), res


def kernel(input, offset, weight, bias):
    out, _ = run_on_hw(input, offset, weight, bias)
    return out


# revision 22
# speedup vs baseline: 48672.4844x; 1.2822x over previous
"""Trainium2 Bass kernel for nn_DeformConv2d_72765335929324.

The module is a dense 3x3 conv (stride 1, pad 1) [B,64,256,256] -> [B,18,256,256]
plus a per-pixel additive `offset` term and a channel bias.

Strategy (per core; batch is sharded 2 images/core across 8 cores):
- conv = 9 taps, each a [cin=64 -> cout=18] matmul over shifted input views.
- PE array packing via tile_position: 2 images on row-groups {0,64} x 4
  output row-pair chunks on col-groups {0,32,64,96} -> 8 concurrent matmul
  streams, 9 accumulating taps each, N=512 (2 output rows) per stream.
- DMA bandwidth scales with partition width, and per-DMA issue costs
  ~0.75us, so all bulk transfers are full-width [128, *] single DMAs:
  input slabs load both images in one 128-partition DMA per 64-row slab;
  offset+bias are pre-added AND pre-permuted on the host into the exact
  SBUF tile layout (one flat [128, 4096] DMA per image+slab, 14 pad
  channels per col group included); the output leaves in the same packed
  layout and is un-permuted on the host.
- W-edge zero-padding is realized by shrinking the matmul N-span per kw tap;
  H-edge padding by shrinking the row span of the first/last chunk taps.
"""

import contextlib
import os
import numpy as np

import concourse.bass as bass
import concourse.tile as tile
import concourse.mybir as mybir
from concourse.vector_clock import ScopedClock
from concourse.bass_utils import run_bass_kernel_spmd

B, CIN, H, W = 16, 64, 256, 256
COUT = 18
COUTP = 32  # cout padded to a 32-wide PE column group
NCORES = 8
BPC = B // NCORES  # images per core
R = 64  # output rows per input slab
NSLAB = H // R
QPS = R // 8  # quad chunks per slab (each quad = 8 rows)
FQ = 2 * W  # free size of one quad chunk (2 rows x 256 cols)
FS = QPS * FQ  # free size of one slab tile

# tap order: full-coverage center tap first so start=True initializes the
# whole PSUM bank region before partial-coverage taps accumulate.
TAPS = [(1, 1), (0, 0), (0, 1), (0, 2), (1, 0), (1, 2), (2, 0), (2, 1), (2, 2)]

# kw -> (src col offset, dst col offset, ncols): zero-pad at W edges is
# realized by shrinking the span instead of padding SBUF.
KW_SPAN = {0: (0, 1, W - 1), 1: (0, 0, W), 2: (1, 0, W - 1)}

DT_NAME = os.environ.get("CONV_DT", "bfloat16")
# timing experiments only: restrict the tap count (wrong results!)
N_TAPS = int(os.environ.get("CONV_TAPS", "9"))


class _TileContext(tile.TileContext):
    """TileContext whose tail drain spreads its semaphore waits over NOPs.

    The stock _drain_and_barrier puts one wait per logical proc on a single
    Drain instruction; the walrus build here rejects instructions carrying
    more than 1-2 sync waits.
    """

    def _drain_and_barrier(self, tick_clock, wait_clock):
        nc = self.nc
        carriers = [nc.sync.nop(nofuse=True) for _ in range(64)]
        drain_inst = nc.sync.drain()
        wait_clock.add_sem_waits(
            drain_inst.ins, ScopedClock({None: tick_clock.global_clock})
        )
        si = drain_inst.ins.sync_info
        waits = list(si.on_wait or []) if si is not None else []
        if len(waits) > 1:
            si.on_wait = waits[:1]
            extra = waits[1:]
            assert len(extra) <= len(carriers)
            for wt, nop in zip(extra, carriers):
                nsi = nop.ins.sync_info
                if nsi is None:
                    nop.ins.sync_info = mybir.SyncInfo(on_wait=[wt], on_update=[])
                else:
                    nsi.on_wait = [wt]
        nc.all_engine_barrier()
        assert self.sems is not None
        popped = nc._tile_sem_poison_stack.pop()
        assert popped is self._sem_poison
        nc.clear_and_free_semaphores(list(self.sems.allocated().values()))
        nc.all_engine_barrier()


def _split_excess_waits(nc):
    """Spill per-instruction semaphore waits onto same-engine NOP carriers.

    Tile's wait assigner attaches up to ~6 waits to one instruction; the
    walrus build here rejects >1 sync wait on engine instructions (>2 on
    EventSemaphore). A NOP that runs just before the instruction on the same
    engine is semantically equivalent (program order on one engine is
    serial). For DMAs, the wait kept in-descriptor is evaluated by the DGE
    without stalling the issuing engine, so keep the freshest (engine-sem)
    wait there and spill the long-satisfied WAR waits on old DMA completions.
    """
    for bb in nc.m.functions[0].blocks:
        new = []
        for inst in bb.instructions:
            si = inst.sync_info
            waits = list(si.on_wait) if si and si.on_wait else []
            cap = 2 if isinstance(inst, mybir.InstEventSemaphore) else 1
            if len(waits) > cap:
                if isinstance(inst, mybir.InstDMACopy):
                    waits.sort(key=lambda w: ((w.ant_name or "").startswith("DMA"),))
                si.on_wait = waits[:cap]
                for w in waits[cap:]:
                    n = mybir.InstNoOp(
                        name=nc.get_next_instruction_name(), ins=[], outs=[]
                    )
                    n.engine = inst.engine
                    n.sync_info = mybir.SyncInfo(on_wait=[w], on_update=[])
                    new.append(n)
            new.append(inst)
        bb.instructions = new


def build_nc(dt_name=DT_NAME, h=H, reps=1):
    dt_in = getattr(mybir.dt, dt_name)
    f32 = mybir.dt.float32
    nslab = h // R
    nc = bass.Bass()
    x = nc.dram_tensor("x", [BPC, CIN, h, W], dt_in, kind="ExternalInput")
    off = nc.dram_tensor("off", [BPC, nslab, 128, FS], f32, kind="ExternalInput")
    wt = nc.dram_tensor("w", [128, len(TAPS) * COUTP], dt_in, kind="ExternalInput")
    y = nc.dram_tensor("y", [BPC, nslab, 128, FS], f32, kind="ExternalOutput")

    nb = 4 if mybir.dt.size(dt_in) <= 2 else 2  # SBUF budget: fp32 input needs
    with _TileContext(nc) as tc:                 # shallower pools
        with (
            tc.tile_pool(name="wpool", bufs=1) as wpool,
            tc.tile_pool(name="slabp", bufs=2) as slabp,
            tc.tile_pool(name="offp", bufs=nb) as offp,
            tc.tile_pool(name="outp", bufs=nb) as outp,
            tc.tile_pool(name="psump", bufs=8, space="PSUM") as psump,
        ):
            w_t = wpool.tile([128, len(TAPS) * COUTP], dt_in, name="w_t")
            nc.sync.dma_start(w_t[:, :], wt[:, :])

            def load_slab(s):
                # slab slot j <-> input row s*R - 1 + j (R+2 slots w/ halo)
                slab = slabp.tile([128, (R + 2) * W], dt_in, name="slab")
                r_lo = max(0, s * R - 1)
                r_hi = min(h, s * R + R + 1)
                slot0 = r_lo - (s * R - 1)
                nc.sync.dma_start(
                    slab[:, slot0 * W : (slot0 + (r_hi - r_lo)) * W],
                    x[:, :, r_lo:r_hi, :],
                )
                offts = []
                for img in range(BPC):
                    off_t = offp.tile([128, FS], f32, name="off_t")
                    nc.sync.dma_start(off_t[:, :], off[img, s, :, :])
                    offts.append(off_t)
                return slab, offts

            loop_ctx = tc.For_i(0, reps) if reps > 1 else contextlib.nullcontext()
            with loop_ctx:
                nxt = load_slab(0)
                for s in range(nslab):
                    slab, offts = nxt
                    if s + 1 < nslab:
                        nxt = load_slab(s + 1)
                    slab3 = [
                        slab[img * 64 : (img + 1) * 64, :].rearrange(
                            "p (r w) -> p r w", w=W
                        )
                        for img in range(BPC)
                    ]
                    outts = []
                    for img in range(BPC):
                        out_t = outp.tile([128, FS], f32, name="out_t")
                        outts.append(out_t)

                    for q in range(QPS):
                        psums = []
                        for img in range(BPC):
                            psum_t = psump.tile([128, FQ], f32, name="psum_t")
                            psums.append(psum_t)

                        # t-major emission: 8 streams (4 col-groups x 2
                        # images) advance through the taps in lockstep.
                        for ti, (kh, kw) in enumerate(TAPS[:N_TAPS] if N_TAPS else []):
                            for c in range(4):
                                for img in range(BPC):
                                    r0 = q * 8 + 2 * c
                                    gr0 = s * R + r0
                                    row_lo, nrows = 0, 2
                                    if gr0 == 0 and kh == 0:
                                        row_lo, nrows = 1, 1
                                    if gr0 == h - 2 and kh == 2:
                                        nrows = 1
                                    src_off, dst_off, ncol = KW_SPAN[kw]
                                    slot = r0 + row_lo + kh
                                    rhs = slab3[img][
                                        :, slot : slot + nrows, src_off : src_off + ncol
                                    ]
                                    out_ap = psums[img][
                                        32 * c : 32 * c + COUTP, :
                                    ].rearrange("p (r w) -> p r w", w=W)[
                                        :,
                                        row_lo : row_lo + nrows,
                                        dst_off : dst_off + ncol,
                                    ]
                                    lhsT = w_t[
                                        img * 64 : (img + 1) * 64,
                                        ti * COUTP : (ti + 1) * COUTP,
                                    ]
                                    nc.tensor.matmul(
                                        out_ap,
                                        lhsT,
                                        rhs,
                                        start=(ti == 0),
                                        stop=(ti == N_TAPS - 1),
                                        tile_position=(img * 64, 32 * c),
                                        # the sim's accumulation-group sanity
                                        # check mis-addresses partition-sliced
                                        # PSUM groups; its per-element
                                        # pending-zero modeling stays active.
                                        skip_group_check=True,
                                    )

                        for img in range(BPC):
                            dst = outts[img][:, q * FQ : (q + 1) * FQ]
                            if N_TAPS:
                                nc.vector.tensor_add(
                                    dst,
                                    psums[img][:, :],
                                    offts[img][:, q * FQ : (q + 1) * FQ],
                                )
                            else:
                                nc.vector.tensor_copy(
                                    dst, offts[img][:, q * FQ : (q + 1) * FQ]
                                )

                    for img in range(BPC):
                        nc.sync.dma_start(y[img, s, :, :], outts[img][:, :])
    _split_excess_waits(nc)
    return nc


def _pack_off(offb, h):
    """[n, 32, h, W] -> [n, nslab, 128, FS] in the SBUF tile layout.

    row r = s*R + q*8 + c*2 + rw maps to partition c*32+ch, free
    q*512 + rw*256 + w.
    """
    nslab = h // R
    v = offb.reshape(offb.shape[0], COUTP, nslab, QPS, 4, 2, W)
    v = v.transpose(0, 2, 4, 1, 3, 5, 6)  # n, s, c, ch, q, rw, w
    return np.ascontiguousarray(v.reshape(offb.shape[0], nslab, 128, FS))


def _unpack_y(y_dev, h):
    """[n, nslab, 128, FS] packed -> [n, COUT, h, W]."""
    n = y_dev.shape[0]
    nslab = h // R
    v = y_dev.reshape(n, nslab, 4, COUTP, QPS, 2, W)
    v = v.transpose(0, 3, 1, 4, 2, 5, 6)  # n, ch, s, q, c, rw, w
    return v.reshape(n, COUTP, h, W)[:, :COUT]


def pack_inputs(input, offset, weight, bias, dt_name=DT_NAME, h=H):
    np_in = mybir.dt.np(getattr(mybir.dt, dt_name))
    input = np.asarray(input, dtype=np.float32)
    offset = np.asarray(offset, dtype=np.float32)
    weight = np.asarray(weight, dtype=np.float32)
    bias = np.asarray(bias, dtype=np.float32)

    nimg = input.shape[0]
    offb = np.zeros((nimg, COUTP, h, W), dtype=np.float32)
    offb[:, :COUT] = offset[:, :COUT, :h] + bias[None, :, None, None]
    off_packed = _pack_off(offb, h)
    w_packed = np.zeros((128, len(TAPS) * COUTP), dtype=np_in)
    for t, (kh, kw) in enumerate(TAPS):
        w_packed[0:64, t * COUTP : t * COUTP + COUT] = weight[:, :, kh, kw].T.astype(
            np_in
        )
    w_packed[64:128] = w_packed[0:64]
    xc = input.astype(np_in)
    in_maps = [
        {
            "x": np.ascontiguousarray(xc[BPC * k : BPC * (k + 1), :, :h]),
            "off": off_packed[BPC * k : BPC * (k + 1)],
            "w": w_packed,
        }
        for k in range(NCORES)
    ]
    return in_maps


_NC_CACHE = {}


def run_on_hw(input, offset, weight, bias, dt_name=DT_NAME, trace=False):
    key = dt_name
    if key not in _NC_CACHE:
        _NC_CACHE[key] = build_nc(dt_name)
    nc = _NC_CACHE[key]
    in_maps = pack_inputs(input, offset, weight, bias, dt_name)
    res = run_bass_kernel_spmd(nc, in_maps, list(range(NCORES)), trace=trace)
    y_dev = np.concatenate([res.results[k]["y"] for k in range(NCORES)], axis=0)
    out = _unpack_y(y_dev, H)
    return np.ascontiguousarray(out.astype(np.float32, copy=False)), res


def kernel(input, offset, weight, bias):
    out, _ = run_on_hw(input, offset, weight, bias)
    return out
